# revision 30
# baseline (speedup 1.0000x reference)
"""Trainium2 Bass kernel for sigmoid-gated multi-head attention.

Reference computation (B=4, F=256, H=8, S=1024):
    qx  = q_input^T          (b, s, f)
    q   = qx @ Wq  -> (b, s, f, h)   [col fi*H + hi]
    k,v = kvx @ Wk / Wv
    attn = sigmoid(sqrt(F) * q.k)    per head
    wv   = attn @ v
    out  = relu(concat_heads(wv) @ Wz)   returned as (b, f, s)

Algebraic restructure (host-side weight folding):
    qkt_h = qx (Wq_h Wk_h^T) kvx^T = qx A_h kvx^T
    out   = relu(sum_h attn_h kvx (Wv_h Wz_h)) = relu(sum_h (attn_h kvx) N_h)
A_h and N_h are tiny 256x256 products computed on the host in fp32.

Per-core MACs 2.684G = the perfect 8-way-sharding ideal, zero
collectives. PE floor 68.3us at the fp16 1-col/cycle rate (measured:
fp8 DoubleRow retires the same 1 col/cycle with 2 K-tiles -> any
precision-preserving fp8 scheme needs hi+lo splits and only breaks
even, so fp16 is optimal here).

Sharding: 8 cores = 4 batches x 2 query-sequence halves; per-core
outputs are disjoint slices of the final output.

Front-window plan (runtime preamble fixed at ~7.2us; DMA triggers cost
~0.65us each, serialized per issuing engine; first ring data ~1.4us
after trigger):
    sync   ring: fr = [A0|qin] packed (384KB, ONE trigger) then n0
    scalar ring: kv_c0 (256KB) then kvt jb0-3 (256KB)
    gpsimd ring: warm memset, kv_c1, kvt jb4-7, then A|N weight pairs
                 for heads 1..7 streamed via pool-reuse deferral
PE warmup starts ~7.4us (gpsimd memset lands early) so the clock is
fully ramped when qa0 fires at ~10.4us. Head-0 qkt opens 4 c0-only
PSUM chains first so late-landing kv_c1 (gpsimd ring) doesn't stall.

Per head (all matmuls [128 x (2|8 chained) x 512], fp16 operands,
fp32 PSUM):
    qa   (fk 2x128, i 512) = A_h^T @ qin           4 mm   (pipelined 1 head ahead)
    qktT (j 8x128, i 512)  = kvin^T_slice @ qa    16 mm -> sigmoid(16x) -> atn
    rawT (fk 2x128, i 512) = sum_jb kvt_slice^T @ atn
                                                  16 mm (2 interleaved chains)
    outT (fo 2x128, i 512) += N_h^T @ rawT         4 mm  (persistent PSUM accum)

Tail: final head's out matmuls ordered so chunk 0 completes early;
ReLUs split by i-half across vector+scalar; 4 output DMAs (64KB each)
issued from sync+gpsimd in parallel for earliest last-byte.
"""

import os
import sys

sys.path.insert(0, "/opt/trn_rl_repo")

import numpy as np

B, F, H, S = 4, 256, 8, 1024
HALF = S // 2  # query columns per core
QTR = HALF // 2
NCORES = 8
P = 128  # partitions

_cache = {}


def _build():
    import concourse.mybir as mybir
    import concourse.tile as tile
    from concourse import bacc

    dt = mybir.dt
    f32 = dt.float32
    f16 = dt.float16
    AF = mybir.ActivationFunctionType

    nc = bacc.Bacc(None, target_bir_lowering=False)

    # head-0 q-side fold + this core's q slice. Separate SBUF tiles:
    # a packed single tile would make qa0's two PE operand streams read
    # the same tile and halves the matmul rate (SBUF port conflict).
    a0_d = nc.dram_tensor("a0", [P, 2, F], f16, kind="ExternalInput")
    qin_d = nc.dram_tensor("qin", [P, 2, HALF], f16, kind="ExternalInput")
    n0_d = nc.dram_tensor("n0", [P, 2, F], f16, kind="ExternalInput")
    # kvin chunk c: [f-low partition, j]; two tensors so each rides its
    # own ring and head-0's c0-only chains can start before c1 lands.
    kv0_d = nc.dram_tensor("kv0", [P, S], f16, kind="ExternalInput")
    kv1_d = nc.dram_tensor("kv1", [P, S], f16, kind="ExternalInput")
    # kvx transposed [j within block, f], split jb 0-3 / 4-7
    kvta_d = nc.dram_tensor("kvta", [P, 4, F], f16, kind="ExternalInput")
    kvtb_d = nc.dram_tensor("kvtb", [P, 4, F], f16, kind="ExternalInput")
    # heads 1..7: [slot 0=A_h rows g | slot 1=N_h rows f][c][col],
    # head-major so each per-head DMA is one contiguous 256KB block
    wh_d = nc.dram_tensor("wh", [H - 1, P, 2, 2, F], f16, kind="ExternalInput")
    # chunk-major: each fo-chunk is one contiguous 128KB block in DRAM
    out_d = nc.dram_tensor("out", [2, P, HALF], f16, kind="ExternalOutput")

    with tile.TileContext(nc) as tc:
        with (
            tc.tile_pool(name="io", bufs=1) as io_pool,
            tc.tile_pool(name="wts", bufs=2) as w_pool,
            tc.tile_pool(name="qa", bufs=2) as qa_pool,
            tc.tile_pool(name="raw", bufs=2) as raw_pool,
            tc.tile_pool(name="attn", bufs=2) as attn_pool,
            tc.tile_pool(name="ps", bufs=6, space="PSUM") as ps_pool,
            tc.tile_pool(name="ops", bufs=1, space="PSUM") as out_ps_pool,
        ):
            a0 = io_pool.tile([P, 2, F], f16, tag="a0")
            qin = io_pool.tile([P, 2, HALF], f16, tag="qin")
            n0 = io_pool.tile([P, 2, F], f16, tag="n0")
            kvin = [
                io_pool.tile([P, S], f16, tag=f"kvin{c}", name=f"kvin{c}")
                for c in range(2)
            ]
            kvt = [
                io_pool.tile([P, 4, F], f16, tag=f"kvt{u}", name=f"kvt{u}")
                for u in range(2)
            ]
            # warm tile is mostly uninitialized: warmup matmuls only keep
            # the PE clock ramped and tolerate garbage (results discarded),
            # so no full memset gates them. Only the first 64 columns --
            # read by the activation wake-ups below, where garbage bit
            # patterns are riskier -- are cleared (tiny gpsimd memset).
            warm = io_pool.tile([P, HALF], dt.bfloat16, tag="warm")
            nc.gpsimd.memset(warm[:, :64], 0.0)

            # front triggers, one engine each (each DIRECT2D costs ~0.65us
            # of issuing-engine time; data starts ~1.4us after trigger)
            nc.sync.dma_start(a0[:], a0_d[:])
            nc.sync.dma_start(qin[:], qin_d[:])
            nc.sync.dma_start(n0[:], n0_d[:])
            nc.scalar.dma_start(kvin[0][:], kv0_d[:])
            nc.scalar.dma_start(kvt[0][:], kvta_d[:])
            nc.gpsimd.dma_start(kvin[1][:], kv1_d[:])
            nc.gpsimd.dma_start(kvt[1][:], kvtb_d[:])

            # heads 1..7 weights stream on gpsimd; bufs=2 pool holds each
            # DMA until head h-2's weights are consumed
            ws = [None] * H

            def fetch_w(h):
                ws[h] = w_pool.tile([P, 2, 2, F], f16, tag="w", name=f"w{h}")
                nc.gpsimd.dma_start(ws[h][:], wh_d[h - 1])

            # hold w1/w2 until kvtb's data is in: their 512KB would
            # otherwise stream right through the critical front window
            wgate = io_pool.tile([P, 16], f16, tag="wgate")
            nc.gpsimd.tensor_copy(wgate[:], kvt[1][:, 0, :16])
            fetch_w(1)
            fetch_w(2)

            # PE pre-warm: fine-grained dummy matmuls bridge until the
            # front DMAs land while keeping the PE clock ramped.
            nwarm = int(os.environ.get("ATTN_NWARM", "32"))
            wps = [
                ps_pool.tile([P, HALF], f32, tag="ps", name=f"wps{i}")
                for i in range(2)
            ]
            for i in range(nwarm):
                nc.tensor.matmul(
                    wps[i % 2][:, :P], warm[:, :P], warm[:, :P],
                    start=True, stop=True,
                )
            # Dependency-free Sigmoid+Relu wake-ups on the junk tile: the
            # FIRST scalar activation must be a Sigmoid so the ACT-table
            # pass loads the sigmoid_and_others set (which also covers
            # Copy/Relu/Identity) up front. Otherwise the first use being
            # a Copy cast loads a copy-table and the sigmoid table load
            # (1.28us) lands mid-body, right before head 0's sigmoids.
            nc.scalar.activation(warm[:, 32:48], warm[:, :16], AF.Sigmoid)
            nc.scalar.activation(warm[:, 48:64], warm[:, :16], AF.Relu)
            # late wake-ups anchored on the warm-ups' psum reads: they
            # execute at bridge-end so both cast engines are recently
            # active when head 0's first casts arrive.
            nc.vector.tensor_copy(warm[:, :16], wps[0][:, :16])
            nc.scalar.activation(warm[:, 16:32], wps[1][:, :16], AF.Copy)

            # persistent accumulators for the folded output projection,
            # one tile per fo-chunk so each chunk's ReLU depends only on
            # its own final matmul (deps are tile-granular for writers)
            out_ps = [
                out_ps_pool.tile([P, HALF], f32, tag=f"out_ps{t}", name=f"out_ps{t}")
                for t in range(2)
            ]

            def a_slice(h, c, t):
                if h == 0:
                    return a0[:, c, P * t : P * (t + 1)]
                return ws[h][:, 0, c, P * t : P * (t + 1)]

            def n_slice(h, c, t):
                if h == 0:
                    return n0[:, c, P * t : P * (t + 1)]
                return ws[h][:, 1, c, P * t : P * (t + 1)]

            qin_ap = qin

            def qa_proj(h):
                """Emit q-side projection qa = A_h^T @ qin for head h."""
                qa = qa_pool.tile([P, 2, HALF], f16, tag="qa", name=f"qa{h}")
                for t in range(2):
                    ps = ps_pool.tile([P, HALF], f32, tag="ps", name=f"psqa{h}{t}")
                    for c in range(2):
                        nc.tensor.matmul(
                            ps[:],
                            a_slice(h, c, t),
                            qin_ap[:, c, :],
                            start=(c == 0),
                            stop=(c == 1),
                        )
                    # split casts across engines to halve qa latency
                    if t == 0:
                        nc.vector.tensor_copy(qa[:, t, :], ps[:])
                    else:
                        nc.scalar.activation(qa[:, t, :], ps[:], AF.Copy)
                return qa

            qa_next = qa_proj(0)
            for h in range(H):
                qa = qa_next

                # qktT (j 8x128, i 512) = kvin_slice^T @ qa; sigmoid on ACT
                atn = attn_pool.tile([P, 8, HALF], f16, tag="atn", name=f"atn{h}")
                pss = {}
                if h == 0:
                    # kv c1 rides the late gpsimd ring: open six c0-only
                    # chains (6 = whole ps pool) before touching c1
                    seq = [(jb, 0) for jb in range(6)]
                    seq += [(jb, 1) for jb in range(6)]
                    seq += [(6, 0), (6, 1), (7, 0), (7, 1)]
                else:
                    seq = [(jb, c) for jb in range(8) for c in range(2)]
                for jb, c in seq:
                    if c == 0:
                        pss[jb] = ps_pool.tile(
                            [P, HALF], f32, tag="ps", name=f"psq{h}{jb}"
                        )
                    nc.tensor.matmul(
                        pss[jb][:],
                        kvin[c][:, P * jb : P * (jb + 1)],
                        qa[:, c, :],
                        start=(c == 0),
                        stop=(c == 1),
                    )
                    if c == 1:
                        nc.scalar.activation(
                            atn[:, jb, :], pss[jb][:], AF.Sigmoid, scale=16.0
                        )

                # fetch head h+2's weights (held by the pool-reuse dep
                # until head h's weights are consumed)
                if 3 <= h + 2 < H:
                    fetch_w(h + 2)

                # rawT (fk 2x128, i 512) = sum_jb kvt_slice^T @ atn_jb.
                # Two chains (fk chunks) interleaved per jb so the PE
                # consumes each sigmoid output ~2x later than a straight
                # chain would.
                raw = raw_pool.tile([P, 2, HALF], f16, tag="raw", name=f"raw{h}")
                rps = [
                    ps_pool.tile([P, HALF], f32, tag="ps", name=f"psr{h}{t}")
                    for t in range(2)
                ]
                # finish the t=0 chain early so its cast runs under the
                # t=1 tail and the N-apply (which consumes raw chunk 0
                # first) never stalls on the serialized vector casts. The
                # last head has no qa_next phase to cover cast latency, so
                # close t=0 six matmuls early there (sigmoids still lead
                # the chain comfortably).
                early = 6 if h == H - 1 else 3
                k = 8 - early
                seq = [(jb, t) for jb in range(k) for t in range(2)]
                seq += [(jb, 0) for jb in range(k, 8)]
                seq += [(jb, 1) for jb in range(k, 8)]
                for jb, t in seq:
                    nc.tensor.matmul(
                        rps[t][:],
                        kvt[jb // 4][:, jb % 4, P * t : P * (t + 1)],
                        atn[:, jb, :],
                        start=(jb == 0),
                        stop=(jb == 7),
                    )
                # both casts on vector: scalar is sigmoid-backlogged here
                nc.vector.tensor_copy(raw[:, 0, :], rps[0][:])
                nc.vector.tensor_copy(raw[:, 1, :], rps[1][:])

                # software-pipeline: next head's qa projection here so the
                # PE has dependency-free work across the head boundary
                if h + 1 < H:
                    qa_next = qa_proj(h + 1)

                # outT (fo 2x128, i 512) += N_h^T @ rawT, persistent accum.
                last = h == H - 1
                if not last:
                    # c-outer: first two matmuls only need raw chunk 0
                    for c in range(2):
                        for t2 in range(2):
                            nc.tensor.matmul(
                                out_ps[t2][:],
                                n_slice(h, c, t2),
                                raw[:, c, :],
                                start=(h == 0 and c == 0),
                                stop=False,
                            )
                else:
                    # chunk 1 completes one matmul early so its ReLU +
                    # DMA (scalar path, slower semaphore) start under the
                    # final chunk-0 matmul
                    for c, t2 in ((0, 1), (0, 0), (1, 1), (1, 0)):
                        nc.tensor.matmul(
                            out_ps[t2][:],
                            n_slice(h, c, t2),
                            raw[:, c, :],
                            start=False,
                            stop=(c == 1),
                        )

            # tail: parallel ReLUs on vector+scalar; chunk 1 (done first)
            # splits its stream across the scalar+sync rings, chunk 0
            # (vector relu, ~40ns start) rides sync. Keep gpsimd out of
            # late DMAs: its dge_drain serializes ~2.5us behind in-flight
            # SWDGE transfers before the exit barrier.
            # separate SBUF tiles per chunk: writers serialize per tile,
            # so sharing one tile would chain the two ReLUs
            out_sb1 = io_pool.tile([P, HALF], f16, tag="out_sb1")
            nc.scalar.activation(out_sb1[:], out_ps[1][:], AF.Relu)
            nc.scalar.dma_start(out_d[1], out_sb1[:])
            out_sb0 = io_pool.tile([P, HALF], f16, tag="out_sb0")
            nc.vector.tensor_relu(out_sb0[:], out_ps[0][:])
            nc.sync.dma_start(out_d[0], out_sb0[:])

    nc.compile()
    return nc


def _get_nc():
    if "nc" not in _cache:
        _cache["nc"] = _build()
    return _cache["nc"]


def _make_in_maps(inputs):
    q_input = np.asarray(inputs["q_input"], dtype=np.float32)
    kv_input = np.asarray(inputs["kv_input"], dtype=np.float32)
    Wq4 = np.asarray(inputs["Wq"], dtype=np.float32).reshape(F, F, H)
    Wk4 = np.asarray(inputs["Wk"], dtype=np.float32).reshape(F, F, H)
    Wv4 = np.asarray(inputs["Wv"], dtype=np.float32).reshape(F, F, H)
    Wz3 = np.asarray(inputs["Wz"], dtype=np.float32).reshape(F, H, F)

    # host-side folds in fp32: A_h = Wq_h Wk_h^T, N_h = Wv_h Wz_h
    A = np.einsum("gdh,fdh->hgf", Wq4, Wk4, optimize=True)  # [H, g, fk]
    N = np.einsum("fdh,dho->hfo", Wv4, Wz3, optimize=True)  # [H, fk, fo]
    # [h, p, c, col] with contraction row = c*128+p
    A_pc = np.ascontiguousarray(
        A.reshape(H, 2, P, F).transpose(0, 2, 1, 3), dtype=np.float16
    )
    N_pc = np.ascontiguousarray(
        N.reshape(H, 2, P, F).transpose(0, 2, 1, 3), dtype=np.float16
    )
    # heads 1..7 packed [h-1, p, slot, c, col]
    WH = np.ascontiguousarray(
        np.stack([A_pc[1:], N_pc[1:]], axis=2), dtype=np.float16
    )  # [H-1, P, 2, 2, F]

    in_maps = []
    for core in range(NCORES):
        b, half = divmod(core, 2)
        qb = q_input[b].reshape(2, P, S)
        qin = np.ascontiguousarray(
            qb[:, :, half * HALF : (half + 1) * HALF].transpose(1, 0, 2),
            dtype=np.float16,
        )
        kvc = kv_input[b].reshape(2, P, S).astype(np.float16)
        # kvt[p, jb, f] = kv_input[b][f, jb*128+p]
        kvt = np.ascontiguousarray(
            kv_input[b].T.reshape(8, P, F).transpose(1, 0, 2), dtype=np.float16
        )
        in_maps.append(
            {
                "a0": A_pc[0],
                "qin": qin,
                "n0": np.ascontiguousarray(N_pc[0]),
                "kv0": np.ascontiguousarray(kvc[0]),
                "kv1": np.ascontiguousarray(kvc[1]),
                "kvta": np.ascontiguousarray(kvt[:, 0:4]),
                "kvtb": np.ascontiguousarray(kvt[:, 4:8]),
                "wh": WH,
            }
        )
    return in_maps


def kernel(q_input, kv_input, Wq, Wk, Wv, Wz, **kw):
    from concourse.bass_utils import run_bass_kernel_spmd

    nc = _get_nc()
    in_maps = _make_in_maps(
        {
            "q_input": q_input,
            "kv_input": kv_input,
            "Wq": Wq,
            "Wk": Wk,
            "Wv": Wv,
            "Wz": Wz,
        }
    )

    res = run_bass_kernel_spmd(nc, in_maps, core_ids=list(range(NCORES)))

    out = np.empty((B, F, S), dtype=np.float32)
    for c in range(NCORES):
        b, half = divmod(c, 2)
        # out dram [chunk, p, i] -> out[b, chunk*128+p, half*512+i]
        o = np.asarray(res.results[c]["out"], dtype=np.float32)  # (2, P, HALF)
        out[b, :, half * HALF : (half + 1) * HALF] = o.reshape(F, HALF)
    return out


# revision 33
# speedup vs baseline: 1.0012x; 1.0012x over previous
"""Trainium2 Bass kernel for sigmoid-gated multi-head attention.

Reference computation (B=4, F=256, H=8, S=1024):
    qx  = q_input^T          (b, s, f)
    q   = qx @ Wq  -> (b, s, f, h)   [col fi*H + hi]
    k,v = kvx @ Wk / Wv
    attn = sigmoid(sqrt(F) * q.k)    per head
    wv   = attn @ v
    out  = relu(concat_heads(wv) @ Wz)   returned as (b, f, s)

Algebraic restructure (host-side weight folding):
    qkt_h = qx (Wq_h Wk_h^T) kvx^T = qx A_h kvx^T
    out   = relu(sum_h attn_h kvx (Wv_h Wz_h)) = relu(sum_h (attn_h kvx) N_h)
A_h and N_h are tiny 256x256 products computed on the host in fp32.

Per-core MACs 2.684G = the perfect 8-way-sharding ideal, zero
collectives. PE floor 68.3us at the fp16 1-col/cycle rate (measured:
fp8 DoubleRow retires the same 1 col/cycle with 2 K-tiles -> any
precision-preserving fp8 scheme needs hi+lo splits and only breaks
even, so fp16 is optimal here).

Sharding: 8 cores = 4 batches x 2 query-sequence halves; per-core
outputs are disjoint slices of the final output.

Front-window plan (runtime preamble fixed at ~7.2us; DMA triggers cost
~0.65us each, serialized per issuing engine; first ring data ~1.4us
after trigger):
    sync   ring: a0 (128KB), qin (256KB), n0 (128KB)
    scalar ring: kv_c0 (256KB) then kvt jb0-3 (256KB)
    gpsimd ring: kv_c1, kvt jb4-7, then A|N weight pairs for heads
                 1..7 (w1/w2 gated behind kvtb's completion so their
                 512KB stays out of the critical front window; later
                 heads held by the bufs=2 pool-reuse dependency)
a0 and qin are separate tiles: packing them into one tile makes qa0's
two PE operand streams read the same SBUF tile at half rate. PE warmup
(junk tile, no init dependency) runs from block entry so the clock is
ramped when qa0 fires at ~10.5-11.5us (data-gated). Head-0 qkt opens
six c0-only PSUM chains first so late-landing kv_c1 (gpsimd ring)
doesn't stall. Dependency-free Sigmoid/Relu wake-ups force both ACT
table loads under warmup (first-use Copy would otherwise trigger a
1.28us sigmoid-table load mid-body).

Per head (all matmuls [128 x (2|8 chained) x 512], fp16 operands,
fp32 PSUM):
    qa   (fk 2x128, i 512) = A_h^T @ qin           4 mm   (pipelined 1 head ahead)
    qktT (j 8x128, i 512)  = kvin^T_slice @ qa    16 mm -> sigmoid(16x) -> atn
    rawT (fk 2x128, i 512) = sum_jb kvt_slice^T @ atn
                                                  16 mm (2 interleaved chains)
    outT (fo 2x128, i 512) += N_h^T @ rawT         4 mm  (persistent PSUM accum)

Tail: final head's out matmuls ordered so chunk 1 completes first
(its scalar-relu path has ~0.8us semaphore latency); per-chunk out_ps
and out_sb tiles keep the two relu+DMA paths independent (writers
serialize per tile); both 128KB output chunks stream concurrently on
the sync and scalar HWDGE rings (never gpsimd: its dge_drain
serializes ~2.5us behind in-flight transfers before the exit
barrier). Measured last-matmul to last-byte: ~2.8us.
"""

import os
import sys

sys.path.insert(0, "/opt/trn_rl_repo")

import numpy as np

B, F, H, S = 4, 256, 8, 1024
HALF = S // 2  # query columns per core
QTR = HALF // 2
NCORES = 8
P = 128  # partitions

_cache = {}


def _build():
    import concourse.mybir as mybir
    import concourse.tile as tile
    from concourse import bacc

    dt = mybir.dt
    f32 = dt.float32
    f16 = dt.float16
    AF = mybir.ActivationFunctionType

    nc = bacc.Bacc(None, target_bir_lowering=False)

    # head-0 q-side fold + this core's q slice. Separate SBUF tiles:
    # a packed single tile would make qa0's two PE operand streams read
    # the same tile and halves the matmul rate (SBUF port conflict).
    a0_d = nc.dram_tensor("a0", [P, 2, F], f16, kind="ExternalInput")
    qin_d = nc.dram_tensor("qin", [P, 2, HALF], f16, kind="ExternalInput")
    n0_d = nc.dram_tensor("n0", [P, 2, F], f16, kind="ExternalInput")
    # kvin chunk c: [f-low partition, j]; two tensors so each rides its
    # own ring and head-0's c0-only chains can start before c1 lands.
    kv0_d = nc.dram_tensor("kv0", [P, S], f16, kind="ExternalInput")
    kv1_d = nc.dram_tensor("kv1", [P, S], f16, kind="ExternalInput")
    # kvx transposed [j within block, f], split jb 0-3 / 4-7
    kvta_d = nc.dram_tensor("kvta", [P, 4, F], f16, kind="ExternalInput")
    kvtb_d = nc.dram_tensor("kvtb", [P, 4, F], f16, kind="ExternalInput")
    # heads 1..7: [slot 0=A_h rows g | slot 1=N_h rows f][c][col],
    # head-major so each per-head DMA is one contiguous 256KB block
    wh_d = nc.dram_tensor("wh", [H - 1, P, 2, 2, F], f16, kind="ExternalInput")
    # chunk-major: each fo-chunk is one contiguous 128KB block in DRAM
    out_d = nc.dram_tensor("out", [2, P, HALF], f16, kind="ExternalOutput")

    with tile.TileContext(nc) as tc:
        with (
            tc.tile_pool(name="io", bufs=1) as io_pool,
            tc.tile_pool(name="wts", bufs=2) as w_pool,
            tc.tile_pool(name="qa", bufs=2) as qa_pool,
            tc.tile_pool(name="raw", bufs=2) as raw_pool,
            tc.tile_pool(name="attn", bufs=2) as attn_pool,
            tc.tile_pool(name="ps", bufs=6, space="PSUM") as ps_pool,
            tc.tile_pool(name="ops", bufs=1, space="PSUM") as out_ps_pool,
        ):
            a0 = io_pool.tile([P, 2, F], f16, tag="a0")
            qin = io_pool.tile([P, 2, HALF], f16, tag="qin")
            n0 = io_pool.tile([P, 2, F], f16, tag="n0")
            kvin = [
                io_pool.tile([P, S], f16, tag=f"kvin{c}", name=f"kvin{c}")
                for c in range(2)
            ]
            kvt = [
                io_pool.tile([P, 4, F], f16, tag=f"kvt{u}", name=f"kvt{u}")
                for u in range(2)
            ]
            # warm tile is mostly uninitialized: warmup matmuls only keep
            # the PE clock ramped and tolerate garbage (results discarded),
            # so no full memset gates them. Only the first 64 columns --
            # read by the activation wake-ups below, where garbage bit
            # patterns are riskier -- are cleared (tiny gpsimd memset).
            warm = io_pool.tile([P, HALF], dt.bfloat16, tag="warm")
            nc.gpsimd.memset(warm[:, :64], 0.0)

            # front triggers, one engine each (each DIRECT2D costs ~0.65us
            # of issuing-engine time; data starts ~1.4us after trigger)
            nc.sync.dma_start(a0[:], a0_d[:])
            nc.sync.dma_start(qin[:], qin_d[:])
            nc.sync.dma_start(n0[:], n0_d[:])
            nc.scalar.dma_start(kvin[0][:], kv0_d[:])
            nc.scalar.dma_start(kvt[0][:], kvta_d[:])
            nc.gpsimd.dma_start(kvin[1][:], kv1_d[:])
            nc.gpsimd.dma_start(kvt[1][:], kvtb_d[:])

            # heads 1..7 weights stream on gpsimd; bufs=2 pool holds each
            # DMA until head h-2's weights are consumed
            ws = [None] * H

            def fetch_w(h):
                ws[h] = w_pool.tile([P, 2, 2, F], f16, tag="w", name=f"w{h}")
                nc.gpsimd.dma_start(ws[h][:], wh_d[h - 1])

            # hold w1/w2 until kvtb's data is in: their 512KB would
            # otherwise stream right through the critical front window
            wgate = io_pool.tile([P, 16], f16, tag="wgate")
            nc.gpsimd.tensor_copy(wgate[:], kvt[1][:, 0, :16])
            fetch_w(1)
            fetch_w(2)

            # PE pre-warm: fine-grained dummy matmuls bridge until the
            # front DMAs land while keeping the PE clock ramped.
            nwarm = int(os.environ.get("ATTN_NWARM", "36"))
            wps = [
                ps_pool.tile([P, HALF], f32, tag="ps", name=f"wps{i}")
                for i in range(2)
            ]
            for i in range(nwarm):
                nc.tensor.matmul(
                    wps[i % 2][:, :P], warm[:, :P], warm[:, :P],
                    start=True, stop=True,
                )
            # Dependency-free Sigmoid+Relu wake-ups on the junk tile: the
            # FIRST scalar activation must be a Sigmoid so the ACT-table
            # pass loads the sigmoid_and_others set (which also covers
            # Copy/Relu/Identity) up front. Otherwise the first use being
            # a Copy cast loads a copy-table and the sigmoid table load
            # (1.28us) lands mid-body, right before head 0's sigmoids.
            nc.scalar.activation(warm[:, 32:48], warm[:, :16], AF.Sigmoid)
            nc.scalar.activation(warm[:, 48:64], warm[:, :16], AF.Relu)
            # late wake-ups anchored on the warm-ups' psum reads: they
            # execute at bridge-end so both cast engines are recently
            # active when head 0's first casts arrive.
            nc.vector.tensor_copy(warm[:, :16], wps[0][:, :16])
            nc.scalar.activation(warm[:, 16:32], wps[1][:, :16], AF.Copy)

            # persistent accumulators for the folded output projection,
            # one tile per fo-chunk so each chunk's ReLU depends only on
            # its own final matmul (deps are tile-granular for writers)
            out_ps = [
                out_ps_pool.tile([P, HALF], f32, tag=f"out_ps{t}", name=f"out_ps{t}")
                for t in range(2)
            ]

            def a_slice(h, c, t):
                if h == 0:
                    return a0[:, c, P * t : P * (t + 1)]
                return ws[h][:, 0, c, P * t : P * (t + 1)]

            def n_slice(h, c, t):
                if h == 0:
                    return n0[:, c, P * t : P * (t + 1)]
                return ws[h][:, 1, c, P * t : P * (t + 1)]

            qin_ap = qin

            def qa_proj(h):
                """Emit q-side projection qa = A_h^T @ qin for head h."""
                qa = qa_pool.tile([P, 2, HALF], f16, tag="qa", name=f"qa{h}")
                for t in range(2):
                    ps = ps_pool.tile([P, HALF], f32, tag="ps", name=f"psqa{h}{t}")
                    for c in range(2):
                        nc.tensor.matmul(
                            ps[:],
                            a_slice(h, c, t),
                            qin_ap[:, c, :],
                            start=(c == 0),
                            stop=(c == 1),
                        )
                    # split casts across engines to halve qa latency
                    if t == 0:
                        nc.vector.tensor_copy(qa[:, t, :], ps[:])
                    else:
                        nc.scalar.activation(qa[:, t, :], ps[:], AF.Copy)
                return qa

            qa_next = qa_proj(0)
            for h in range(H):
                qa = qa_next

                # qktT (j 8x128, i 512) = kvin_slice^T @ qa; sigmoid on ACT
                atn = attn_pool.tile([P, 8, HALF], f16, tag="atn", name=f"atn{h}")
                pss = {}
                if h == 0:
                    # kv c1 rides the late gpsimd ring: open six c0-only
                    # chains (6 = whole ps pool) before touching c1
                    seq = [(jb, 0) for jb in range(6)]
                    seq += [(jb, 1) for jb in range(6)]
                    seq += [(6, 0), (6, 1), (7, 0), (7, 1)]
                else:
                    seq = [(jb, c) for jb in range(8) for c in range(2)]
                for jb, c in seq:
                    if c == 0:
                        pss[jb] = ps_pool.tile(
                            [P, HALF], f32, tag="ps", name=f"psq{h}{jb}"
                        )
                    nc.tensor.matmul(
                        pss[jb][:],
                        kvin[c][:, P * jb : P * (jb + 1)],
                        qa[:, c, :],
                        start=(c == 0),
                        stop=(c == 1),
                    )
                    if c == 1:
                        nc.scalar.activation(
                            atn[:, jb, :], pss[jb][:], AF.Sigmoid, scale=16.0
                        )

                # fetch head h+2's weights (held by the pool-reuse dep
                # until head h's weights are consumed)
                if 3 <= h + 2 < H:
                    fetch_w(h + 2)

                # rawT (fk 2x128, i 512) = sum_jb kvt_slice^T @ atn_jb.
                # Two chains (fk chunks) interleaved per jb so the PE
                # consumes each sigmoid output ~2x later than a straight
                # chain would.
                raw = raw_pool.tile([P, 2, HALF], f16, tag="raw", name=f"raw{h}")
                rps = [
                    ps_pool.tile([P, HALF], f32, tag="ps", name=f"psr{h}{t}")
                    for t in range(2)
                ]
                # finish the t=0 chain early so its cast runs under the
                # t=1 tail and the N-apply (which consumes raw chunk 0
                # first) never stalls on the serialized vector casts. The
                # last head has no qa_next phase to cover cast latency, so
                # close t=0 six matmuls early there (sigmoids still lead
                # the chain comfortably).
                early = 6 if h == H - 1 else 3
                k = 8 - early
                seq = [(jb, t) for jb in range(k) for t in range(2)]
                seq += [(jb, 0) for jb in range(k, 8)]
                seq += [(jb, 1) for jb in range(k, 8)]
                for jb, t in seq:
                    nc.tensor.matmul(
                        rps[t][:],
                        kvt[jb // 4][:, jb % 4, P * t : P * (t + 1)],
                        atn[:, jb, :],
                        start=(jb == 0),
                        stop=(jb == 7),
                    )
                # both casts on vector: scalar is sigmoid-backlogged here
                nc.vector.tensor_copy(raw[:, 0, :], rps[0][:])
                nc.vector.tensor_copy(raw[:, 1, :], rps[1][:])

                # software-pipeline: next head's qa projection here so the
                # PE has dependency-free work across the head boundary
                if h + 1 < H:
                    qa_next = qa_proj(h + 1)

                # outT (fo 2x128, i 512) += N_h^T @ rawT, persistent accum.
                last = h == H - 1
                if not last:
                    # c-outer: first two matmuls only need raw chunk 0
                    for c in range(2):
                        for t2 in range(2):
                            nc.tensor.matmul(
                                out_ps[t2][:],
                                n_slice(h, c, t2),
                                raw[:, c, :],
                                start=(h == 0 and c == 0),
                                stop=False,
                            )
                else:
                    # chunk 1 completes one matmul early so its ReLU +
                    # DMA (scalar path, slower semaphore) start under the
                    # final chunk-0 matmul
                    for c, t2 in ((0, 1), (0, 0), (1, 1), (1, 0)):
                        nc.tensor.matmul(
                            out_ps[t2][:],
                            n_slice(h, c, t2),
                            raw[:, c, :],
                            start=False,
                            stop=(c == 1),
                        )

            # tail: parallel ReLUs on vector+scalar; chunk 1 (done first)
            # splits its stream across the scalar+sync rings, chunk 0
            # (vector relu, ~40ns start) rides sync. Keep gpsimd out of
            # late DMAs: its dge_drain serializes ~2.5us behind in-flight
            # SWDGE transfers before the exit barrier.
            # separate SBUF tiles per chunk: writers serialize per tile,
            # so sharing one tile would chain the two ReLUs
            out_sb1 = io_pool.tile([P, HALF], f16, tag="out_sb1")
            nc.scalar.activation(out_sb1[:], out_ps[1][:], AF.Relu)
            nc.scalar.dma_start(out_d[1], out_sb1[:])
            out_sb0 = io_pool.tile([P, HALF], f16, tag="out_sb0")
            nc.vector.tensor_relu(out_sb0[:], out_ps[0][:])
            nc.sync.dma_start(out_d[0], out_sb0[:])

    nc.compile()
    return nc


def _get_nc():
    if "nc" not in _cache:
        _cache["nc"] = _build()
    return _cache["nc"]


def _make_in_maps(inputs):
    q_input = np.asarray(inputs["q_input"], dtype=np.float32)
    kv_input = np.asarray(inputs["kv_input"], dtype=np.float32)
    Wq4 = np.asarray(inputs["Wq"], dtype=np.float32).reshape(F, F, H)
    Wk4 = np.asarray(inputs["Wk"], dtype=np.float32).reshape(F, F, H)
    Wv4 = np.asarray(inputs["Wv"], dtype=np.float32).reshape(F, F, H)
    Wz3 = np.asarray(inputs["Wz"], dtype=np.float32).reshape(F, H, F)

    # host-side folds in fp32: A_h = Wq_h Wk_h^T, N_h = Wv_h Wz_h
    A = np.einsum("gdh,fdh->hgf", Wq4, Wk4, optimize=True)  # [H, g, fk]
    N = np.einsum("fdh,dho->hfo", Wv4, Wz3, optimize=True)  # [H, fk, fo]
    # [h, p, c, col] with contraction row = c*128+p
    A_pc = np.ascontiguousarray(
        A.reshape(H, 2, P, F).transpose(0, 2, 1, 3), dtype=np.float16
    )
    N_pc = np.ascontiguousarray(
        N.reshape(H, 2, P, F).transpose(0, 2, 1, 3), dtype=np.float16
    )
    # heads 1..7 packed [h-1, p, slot, c, col]
    WH = np.ascontiguousarray(
        np.stack([A_pc[1:], N_pc[1:]], axis=2), dtype=np.float16
    )  # [H-1, P, 2, 2, F]

    in_maps = []
    for core in range(NCORES):
        b, half = divmod(core, 2)
        qb = q_input[b].reshape(2, P, S)
        qin = np.ascontiguousarray(
            qb[:, :, half * HALF : (half + 1) * HALF].transpose(1, 0, 2),
            dtype=np.float16,
        )
        kvc = kv_input[b].reshape(2, P, S).astype(np.float16)
        # kvt[p, jb, f] = kv_input[b][f, jb*128+p]
        kvt = np.ascontiguousarray(
            kv_input[b].T.reshape(8, P, F).transpose(1, 0, 2), dtype=np.float16
        )
        in_maps.append(
            {
                "a0": A_pc[0],
                "qin": qin,
                "n0": np.ascontiguousarray(N_pc[0]),
                "kv0": np.ascontiguousarray(kvc[0]),
                "kv1": np.ascontiguousarray(kvc[1]),
                "kvta": np.ascontiguousarray(kvt[:, 0:4]),
                "kvtb": np.ascontiguousarray(kvt[:, 4:8]),
                "wh": WH,
            }
        )
    return in_maps


def kernel(q_input, kv_input, Wq, Wk, Wv, Wz, **kw):
    from concourse.bass_utils import run_bass_kernel_spmd

    nc = _get_nc()
    in_maps = _make_in_maps(
        {
            "q_input": q_input,
            "kv_input": kv_input,
            "Wq": Wq,
            "Wk": Wk,
            "Wv": Wv,
            "Wz": Wz,
        }
    )

    res = run_bass_kernel_spmd(nc, in_maps, core_ids=list(range(NCORES)))

    out = np.empty((B, F, S), dtype=np.float32)
    for c in range(NCORES):
        b, half = divmod(c, 2)
        # out dram [chunk, p, i] -> out[b, chunk*128+p, half*512+i]
        o = np.asarray(res.results[c]["out"], dtype=np.float32)  # (2, P, HALF)
        out[b, :, half * HALF : (half + 1) * HALF] = o.reshape(F, HALF)
    return out


# revision 36
# speedup vs baseline: 1.0294x; 1.0282x over previous
"""Trainium2 Bass kernel for sigmoid-gated multi-head attention.

Reference computation (B=4, F=256, H=8, S=1024):
    qx  = q_input^T          (b, s, f)
    q   = qx @ Wq  -> (b, s, f, h)   [col fi*H + hi]
    k,v = kvx @ Wk / Wv
    attn = sigmoid(sqrt(F) * q.k)    per head
    wv   = attn @ v
    out  = relu(concat_heads(wv) @ Wz)   returned as (b, f, s)

Algebraic restructure (host-side weight folding):
    qkt_h = qx (Wq_h Wk_h^T) kvx^T = qx A_h kvx^T
    out   = relu(sum_h attn_h kvx (Wv_h Wz_h)) = relu(sum_h (attn_h kvx) N_h)
A_h and N_h are tiny 256x256 products computed on the host in fp32.

Per-core MACs 2.684G = the perfect 8-way-sharding ideal, zero
collectives. PE floor 68.3us at the fp16 1-col/cycle rate (measured:
fp8 DoubleRow retires the same 1 col/cycle with 2 K-tiles -> any
precision-preserving fp8 scheme needs hi+lo splits and only breaks
even, so fp16 is optimal here).

Sharding: 8 cores = 4 batches x 2 query-sequence halves; per-core
outputs are disjoint slices of the final output.

Front-window plan (runtime preamble fixed at ~7.2us; DMA triggers cost
~0.65us each, serialized per issuing engine; first ring data ~1.4us
after trigger):
    sync   ring: a0 (128KB), qin (256KB), n0 (128KB)
    scalar ring: kv_c0 (256KB) then kvt jb0-3 (256KB)
    gpsimd ring: kv_c1, kvt jb4-7, then A|N weight pairs for heads
                 1..7 (w1/w2 gated behind kvtb's completion so their
                 512KB stays out of the critical front window; later
                 heads held by the bufs=2 pool-reuse dependency)
a0 and qin are separate tiles: packing them into one tile makes qa0's
two PE operand streams read the same SBUF tile at half rate. PE warmup
(junk tile, no init dependency) runs from block entry so the clock is
ramped when qa0 fires at ~10.5-11.5us (data-gated). Head-0 qkt opens
six c0-only PSUM chains first so late-landing kv_c1 (gpsimd ring)
doesn't stall. Dependency-free Sigmoid/Relu wake-ups force both ACT
table loads under warmup (first-use Copy would otherwise trigger a
1.28us sigmoid-table load mid-body).

Per head (all matmuls [128 x (2|8 chained) x 512], fp16 operands,
fp32 PSUM):
    qa   (fk 2x128, i 512) = A_h^T @ qin           4 mm   (pipelined 1 head ahead)
    qktT (j 8x128, i 512)  = kvin^T_slice @ qa    16 mm -> sigmoid(16x) -> atn
    rawT (fk 2x128, i 512) = sum_jb kvt_slice^T @ atn
                                                  16 mm (2 interleaved chains)
    outT (fo 2x128, i 512) += N_h^T @ rawT         4 mm  (persistent PSUM accum)

Tail: final head's out matmuls ordered so chunk 1 completes first
(its scalar-relu path has ~0.8us semaphore latency); per-chunk out_ps
and out_sb tiles keep the two relu+DMA paths independent (writers
serialize per tile); both 128KB output chunks stream concurrently on
the sync and scalar HWDGE rings (never gpsimd: its dge_drain
serializes ~2.5us behind in-flight transfers before the exit
barrier). Measured last-matmul to last-byte: ~2.8us.
"""

import os
import sys

sys.path.insert(0, "/opt/trn_rl_repo")

import numpy as np

B, F, H, S = 4, 256, 8, 1024
HALF = S // 2  # query columns per core
QTR = HALF // 2
NCORES = 8
P = 128  # partitions

_cache = {}


def _build():
    import concourse.mybir as mybir
    import concourse.tile as tile
    from concourse import bacc

    dt = mybir.dt
    f32 = dt.float32
    f16 = dt.float16
    AF = mybir.ActivationFunctionType

    nc = bacc.Bacc(None, target_bir_lowering=False)

    # head-0 q-side fold + this core's q slice. Separate SBUF tiles:
    # a packed single tile would make qa0's two PE operand streams read
    # the same tile and halves the matmul rate (SBUF port conflict).
    a0_d = nc.dram_tensor("a0", [P, 2, F], f16, kind="ExternalInput")
    qin_d = nc.dram_tensor("qin", [P, 2, HALF], f16, kind="ExternalInput")
    n0_d = nc.dram_tensor("n0", [P, 2, F], f16, kind="ExternalInput")
    # kvin chunk c: [f-low partition, j]; two tensors so each rides its
    # own ring and head-0's c0-only chains can start before c1 lands.
    kv0_d = nc.dram_tensor("kv0", [P, S], f16, kind="ExternalInput")
    kv1_d = nc.dram_tensor("kv1", [P, S], f16, kind="ExternalInput")
    # kvx transposed [j within block, f], split jb 0-3 / 4-7
    kvta_d = nc.dram_tensor("kvta", [P, 4, F], f16, kind="ExternalInput")
    kvtb_d = nc.dram_tensor("kvtb", [P, 4, F], f16, kind="ExternalInput")
    # heads 1..7: [slot 0=A_h rows g | slot 1=N_h rows f][c][col],
    # head-major so each per-head DMA is one contiguous 256KB block
    wh_d = nc.dram_tensor("wh", [H - 1, P, 2, 2, F], f16, kind="ExternalInput")
    # chunk-major: each fo-chunk is one contiguous 128KB block in DRAM
    out_d = nc.dram_tensor("out", [2, P, HALF], f16, kind="ExternalOutput")

    with tile.TileContext(nc) as tc:
        with (
            tc.tile_pool(name="io", bufs=1) as io_pool,
            tc.tile_pool(name="wts", bufs=2) as w_pool,
            tc.tile_pool(name="qa", bufs=2) as qa_pool,
            tc.tile_pool(name="raw", bufs=2) as raw_pool,
            tc.tile_pool(name="attn", bufs=2) as attn_pool,
            tc.tile_pool(name="ps", bufs=6, space="PSUM") as ps_pool,
            tc.tile_pool(name="ops", bufs=1, space="PSUM") as out_ps_pool,
        ):
            a0 = io_pool.tile([P, 2, F], f16, tag="a0")
            qin = io_pool.tile([P, 2, HALF], f16, tag="qin")
            n0 = io_pool.tile([P, 2, F], f16, tag="n0")
            kvin = [
                io_pool.tile([P, S], f16, tag=f"kvin{c}", name=f"kvin{c}")
                for c in range(2)
            ]
            kvt = [
                io_pool.tile([P, 4, F], f16, tag=f"kvt{u}", name=f"kvt{u}")
                for u in range(2)
            ]
            # warm tile is mostly uninitialized: warmup matmuls only keep
            # the PE clock ramped and tolerate garbage (results discarded),
            # so no full memset gates them. Only the first 64 columns --
            # read by the activation wake-ups below, where garbage bit
            # patterns are riskier -- are cleared (tiny gpsimd memset).
            warm = io_pool.tile([P, HALF], dt.bfloat16, tag="warm")
            nc.gpsimd.memset(warm[:, :64], 0.0)

            # front triggers, one engine each (each DIRECT2D costs ~0.65us
            # of issuing-engine time; data starts ~1.4us after trigger)
            nc.sync.dma_start(a0[:], a0_d[:])
            nc.sync.dma_start(qin[:], qin_d[:])
            nc.sync.dma_start(n0[:], n0_d[:])
            nc.scalar.dma_start(kvin[0][:], kv0_d[:])
            nc.scalar.dma_start(kvt[0][:], kvta_d[:])
            nc.gpsimd.dma_start(kvin[1][:], kv1_d[:])
            nc.gpsimd.dma_start(kvt[1][:], kvtb_d[:])

            # heads 1..7 weights stream on gpsimd; bufs=2 pool holds each
            # DMA until head h-2's weights are consumed
            ws = [None] * H

            def fetch_w(h):
                ws[h] = w_pool.tile([P, 2, 2, F], f16, tag="w", name=f"w{h}")
                nc.gpsimd.dma_start(ws[h][:], wh_d[h - 1])

            fetch_w(1)
            fetch_w(2)

            # PE pre-warm: fine-grained dummy matmuls bridge until the
            # front DMAs land while keeping the PE clock ramped.
            nwarm = int(os.environ.get("ATTN_NWARM", "40"))
            wps = [
                ps_pool.tile([P, HALF], f32, tag="ps", name=f"wps{i}")
                for i in range(2)
            ]
            for i in range(nwarm):
                nc.tensor.matmul(
                    wps[i % 2][:, :P], warm[:, :P], warm[:, :P],
                    start=True, stop=True,
                )
            # Dependency-free Sigmoid+Relu wake-ups on the junk tile: the
            # FIRST scalar activation must be a Sigmoid so the ACT-table
            # pass loads the sigmoid_and_others set (which also covers
            # Copy/Relu/Identity) up front. Otherwise the first use being
            # a Copy cast loads a copy-table and the sigmoid table load
            # (1.28us) lands mid-body, right before head 0's sigmoids.
            nc.scalar.activation(warm[:, 32:48], warm[:, :16], AF.Sigmoid)
            nc.scalar.activation(warm[:, 48:64], warm[:, :16], AF.Relu)
            # late wake-ups anchored on the warm-ups' psum reads: they
            # execute at bridge-end so both cast engines are recently
            # active when head 0's first casts arrive.
            nc.vector.tensor_copy(warm[:, :16], wps[0][:, :16])
            nc.scalar.activation(warm[:, 16:32], wps[1][:, :16], AF.Copy)

            # persistent accumulators for the folded output projection,
            # one tile per fo-chunk so each chunk's ReLU depends only on
            # its own final matmul (deps are tile-granular for writers)
            out_ps = [
                out_ps_pool.tile([P, HALF], f32, tag=f"out_ps{t}", name=f"out_ps{t}")
                for t in range(2)
            ]

            def a_slice(h, c, t):
                if h == 0:
                    return a0[:, c, P * t : P * (t + 1)]
                return ws[h][:, 0, c, P * t : P * (t + 1)]

            def n_slice(h, c, t):
                if h == 0:
                    return n0[:, c, P * t : P * (t + 1)]
                return ws[h][:, 1, c, P * t : P * (t + 1)]

            qin_ap = qin

            def qa_proj(h):
                """Emit q-side projection qa = A_h^T @ qin for head h."""
                qa = qa_pool.tile([P, 2, HALF], f16, tag="qa", name=f"qa{h}")
                for t in range(2):
                    ps = ps_pool.tile([P, HALF], f32, tag="ps", name=f"psqa{h}{t}")
                    for c in range(2):
                        nc.tensor.matmul(
                            ps[:],
                            a_slice(h, c, t),
                            qin_ap[:, c, :],
                            start=(c == 0),
                            stop=(c == 1),
                        )
                    # split casts across engines to halve qa latency
                    if t == 0:
                        nc.vector.tensor_copy(qa[:, t, :], ps[:])
                    else:
                        nc.scalar.activation(qa[:, t, :], ps[:], AF.Copy)
                return qa

            qa_next = qa_proj(0)
            for h in range(H):
                qa = qa_next

                # qktT (j 8x128, i 512) = kvin_slice^T @ qa; sigmoid on ACT
                atn = attn_pool.tile([P, 8, HALF], f16, tag="atn", name=f"atn{h}")
                pss = {}
                if h == 0:
                    # kv c1 rides the late gpsimd ring: open six c0-only
                    # chains (6 = whole ps pool) before touching c1
                    seq = [(jb, 0) for jb in range(6)]
                    seq += [(jb, 1) for jb in range(6)]
                    seq += [(6, 0), (6, 1), (7, 0), (7, 1)]
                else:
                    seq = [(jb, c) for jb in range(8) for c in range(2)]
                for jb, c in seq:
                    if c == 0:
                        pss[jb] = ps_pool.tile(
                            [P, HALF], f32, tag="ps", name=f"psq{h}{jb}"
                        )
                    nc.tensor.matmul(
                        pss[jb][:],
                        kvin[c][:, P * jb : P * (jb + 1)],
                        qa[:, c, :],
                        start=(c == 0),
                        stop=(c == 1),
                    )
                    if c == 1:
                        nc.scalar.activation(
                            atn[:, jb, :], pss[jb][:], AF.Sigmoid, scale=16.0
                        )

                # fetch head h+2's weights (held by the pool-reuse dep
                # until head h's weights are consumed)
                if 3 <= h + 2 < H:
                    fetch_w(h + 2)

                # rawT (fk 2x128, i 512) = sum_jb kvt_slice^T @ atn_jb.
                # Two chains (fk chunks) interleaved per jb so the PE
                # consumes each sigmoid output ~2x later than a straight
                # chain would.
                raw = raw_pool.tile([P, 2, HALF], f16, tag="raw", name=f"raw{h}")
                rps = [
                    ps_pool.tile([P, HALF], f32, tag="ps", name=f"psr{h}{t}")
                    for t in range(2)
                ]
                # finish the t=0 chain 3 matmuls early so its cast runs
                # under the t=1 tail and the N-apply (which consumes raw
                # chunk 0 first) never stalls on the serialized vector
                # casts. Sigmoid jb7 still lands ~0.7us before (7,0).
                seq = [(jb, t) for jb in range(5) for t in range(2)]
                seq += [(5, 0), (6, 0), (7, 0), (5, 1), (6, 1), (7, 1)]
                for jb, t in seq:
                    nc.tensor.matmul(
                        rps[t][:],
                        kvt[jb // 4][:, jb % 4, P * t : P * (t + 1)],
                        atn[:, jb, :],
                        start=(jb == 0),
                        stop=(jb == 7),
                    )
                # both casts on vector: scalar is sigmoid-backlogged here
                nc.vector.tensor_copy(raw[:, 0, :], rps[0][:])
                nc.vector.tensor_copy(raw[:, 1, :], rps[1][:])

                # software-pipeline: next head's qa projection here so the
                # PE has dependency-free work across the head boundary
                if h + 1 < H:
                    qa_next = qa_proj(h + 1)

                # outT (fo 2x128, i 512) += N_h^T @ rawT, persistent accum.
                last = h == H - 1
                if not last:
                    # c-outer: first two matmuls only need raw chunk 0
                    for c in range(2):
                        for t2 in range(2):
                            nc.tensor.matmul(
                                out_ps[t2][:],
                                n_slice(h, c, t2),
                                raw[:, c, :],
                                start=(h == 0 and c == 0),
                                stop=False,
                            )
                else:
                    # chunk 1 completes one matmul early so its ReLU +
                    # DMA (scalar path, slower semaphore) start under the
                    # final chunk-0 matmul
                    for c, t2 in ((0, 1), (0, 0), (1, 1), (1, 0)):
                        nc.tensor.matmul(
                            out_ps[t2][:],
                            n_slice(h, c, t2),
                            raw[:, c, :],
                            start=False,
                            stop=(c == 1),
                        )

            # tail: parallel ReLUs on vector+scalar; chunk 1 (done first)
            # splits its stream across the scalar+sync rings, chunk 0
            # (vector relu, ~40ns start) rides sync. Keep gpsimd out of
            # late DMAs: its dge_drain serializes ~2.5us behind in-flight
            # SWDGE transfers before the exit barrier.
            # separate SBUF tiles per chunk: writers serialize per tile,
            # so sharing one tile would chain the two ReLUs
            out_sb1 = io_pool.tile([P, HALF], f16, tag="out_sb1")
            nc.scalar.activation(out_sb1[:], out_ps[1][:], AF.Relu)
            nc.scalar.dma_start(out_d[1], out_sb1[:])
            out_sb0 = io_pool.tile([P, HALF], f16, tag="out_sb0")
            nc.vector.tensor_relu(out_sb0[:], out_ps[0][:])
            nc.sync.dma_start(out_d[0], out_sb0[:])

    nc.compile()
    return nc


def _get_nc():
    if "nc" not in _cache:
        _cache["nc"] = _build()
    return _cache["nc"]


def _make_in_maps(inputs):
    q_input = np.asarray(inputs["q_input"], dtype=np.float32)
    kv_input = np.asarray(inputs["kv_input"], dtype=np.float32)
    Wq4 = np.asarray(inputs["Wq"], dtype=np.float32).reshape(F, F, H)
    Wk4 = np.asarray(inputs["Wk"], dtype=np.float32).reshape(F, F, H)
    Wv4 = np.asarray(inputs["Wv"], dtype=np.float32).reshape(F, F, H)
    Wz3 = np.asarray(inputs["Wz"], dtype=np.float32).reshape(F, H, F)

    # host-side folds in fp32: A_h = Wq_h Wk_h^T, N_h = Wv_h Wz_h
    A = np.einsum("gdh,fdh->hgf", Wq4, Wk4, optimize=True)  # [H, g, fk]
    N = np.einsum("fdh,dho->hfo", Wv4, Wz3, optimize=True)  # [H, fk, fo]
    # [h, p, c, col] with contraction row = c*128+p
    A_pc = np.ascontiguousarray(
        A.reshape(H, 2, P, F).transpose(0, 2, 1, 3), dtype=np.float16
    )
    N_pc = np.ascontiguousarray(
        N.reshape(H, 2, P, F).transpose(0, 2, 1, 3), dtype=np.float16
    )
    # heads 1..7 packed [h-1, p, slot, c, col]
    WH = np.ascontiguousarray(
        np.stack([A_pc[1:], N_pc[1:]], axis=2), dtype=np.float16
    )  # [H-1, P, 2, 2, F]

    in_maps = []
    for core in range(NCORES):
        b, half = divmod(core, 2)
        qb = q_input[b].reshape(2, P, S)
        qin = np.ascontiguousarray(
            qb[:, :, half * HALF : (half + 1) * HALF].transpose(1, 0, 2),
            dtype=np.float16,
        )
        kvc = kv_input[b].reshape(2, P, S).astype(np.float16)
        # kvt[p, jb, f] = kv_input[b][f, jb*128+p]
        kvt = np.ascontiguousarray(
            kv_input[b].T.reshape(8, P, F).transpose(1, 0, 2), dtype=np.float16
        )
        in_maps.append(
            {
                "a0": A_pc[0],
                "qin": qin,
                "n0": np.ascontiguousarray(N_pc[0]),
                "kv0": np.ascontiguousarray(kvc[0]),
                "kv1": np.ascontiguousarray(kvc[1]),
                "kvta": np.ascontiguousarray(kvt[:, 0:4]),
                "kvtb": np.ascontiguousarray(kvt[:, 4:8]),
                "wh": WH,
            }
        )
    return in_maps


def kernel(q_input, kv_input, Wq, Wk, Wv, Wz, **kw):
    from concourse.bass_utils import run_bass_kernel_spmd

    nc = _get_nc()
    in_maps = _make_in_maps(
        {
            "q_input": q_input,
            "kv_input": kv_input,
            "Wq": Wq,
            "Wk": Wk,
            "Wv": Wv,
            "Wz": Wz,
        }
    )

    res = run_bass_kernel_spmd(nc, in_maps, core_ids=list(range(NCORES)))

    out = np.empty((B, F, S), dtype=np.float32)
    for c in range(NCORES):
        b, half = divmod(c, 2)
        # out dram [chunk, p, i] -> out[b, chunk*128+p, half*512+i]
        o = np.asarray(res.results[c]["out"], dtype=np.float32)  # (2, P, HALF)
        out[b, :, half * HALF : (half + 1) * HALF] = o.reshape(F, HALF)
    return out


# revision 46
# speedup vs baseline: 1.0560x; 1.0258x over previous
"""Trainium2 Bass kernel for sigmoid-gated multi-head attention.

Reference computation (B=4, F=256, H=8, S=1024):
    qx  = q_input^T          (b, s, f)
    q   = qx @ Wq  -> (b, s, f, h)   [col fi*H + hi]
    k,v = kvx @ Wk / Wv
    attn = sigmoid(sqrt(F) * q.k)    per head
    wv   = attn @ v
    out  = relu(concat_heads(wv) @ Wz)   returned as (b, f, s)

Algebraic restructure (host-side weight folding):
    qkt_h = qx (Wq_h Wk_h^T) kvx^T = qx A_h kvx^T
    out   = relu(sum_h attn_h kvx (Wv_h Wz_h)) = relu(sum_h (attn_h kvx) N_h)
A_h and N_h are tiny 256x256 products computed on the host in fp32.

Per-core MACs 2.684G = the perfect 8-way-sharding ideal, zero
collectives. PE floor 68.3us at the fp16 1-col/cycle rate (measured:
fp8 DoubleRow retires the same 1 col/cycle with 2 K-tiles -> any
precision-preserving fp8 scheme needs hi+lo splits and only breaks
even, so fp16 is optimal here).

Sharding: 8 cores = 4 batches x 2 query-sequence halves; per-core
outputs are disjoint slices of the final output.

Front-window plan (runtime preamble fixed at ~7.2us; DMA triggers cost
~0.65us each, serialized per issuing engine; first ring data ~1.4us
after trigger):
    sync   ring: fr = [A0|qin] packed (384KB, one trigger), then n0
    scalar ring: kv_c0 (256KB) then kvt jb0-3 (256KB)
    gpsimd ring: kv_c1, kvt jb4-7, w1, w2, then A|N weight pairs for
                 heads 3..7 held by the bufs=2 pool-reuse dependency
                 until head h-2's weights are consumed
PE warmup (junk tile, deliberately uninitialized so nothing gates it)
runs from block entry so the clock is ramped when qa0 fires at
~10.5-11.5us (data-gated). Head-0 qkt opens six c0-only PSUM chains
first so late-landing kv_c1 (gpsimd ring) doesn't stall.

Per head (all matmuls [128 x (2|8 chained) x 512], fp16 operands,
fp32 PSUM):
    qa   (fk 2x128, i 512) = A_h^T @ qin           4 mm   (pipelined 1 head ahead)
    qktT (j 8x128, i 512)  = kvin^T_slice @ qa    16 mm -> sigmoid(16x) -> atn
    rawT (fk 2x128, i 512) = sum_jb kvt_slice^T @ atn
                                                  16 mm (2 interleaved chains)
    outT (fo 2x128, i 512) += N_h^T @ rawT         4 mm  (persistent PSUM accum)

Tail: final head's out matmuls ordered so chunk 1 completes first
(its scalar-relu path has ~0.8us semaphore latency); per-chunk out_ps
and out_sb tiles keep the two relu+DMA paths independent (writers
serialize per tile); both 128KB output chunks stream concurrently on
the sync and scalar HWDGE rings (never gpsimd: its dge_drain
serializes ~2.5us behind in-flight transfers before the exit
barrier). Measured last-matmul to last-byte: ~2.8us.
"""

import os
import sys

sys.path.insert(0, "/opt/trn_rl_repo")

import numpy as np

B, F, H, S = 4, 256, 8, 1024
HALF = S // 2  # query columns per core
QTR = HALF // 2
NCORES = 8
P = 128  # partitions

_cache = {}


def _build():
    import concourse.mybir as mybir
    import concourse.tile as tile
    from concourse import bacc

    dt = mybir.dt
    f32 = dt.float32
    f16 = dt.float16
    AF = mybir.ActivationFunctionType

    nc = bacc.Bacc(None, target_bir_lowering=False)

    # fr[p, c, 0:F]      = A0[c*128+p, :]   (head-0 q-side fold)
    # fr[p, c, F:F+HALF] = qin[c*128+p, :]  (this core's q slice)
    fr_d = nc.dram_tensor("fr", [P, 2, F + HALF], f16, kind="ExternalInput")
    n0_d = nc.dram_tensor("n0", [P, 2, F], f16, kind="ExternalInput")
    # kvin chunk c: [f-low partition, j]; two tensors so each rides its
    # own ring and head-0's c0-only chains can start before c1 lands.
    kv0_d = nc.dram_tensor("kv0", [P, S], f16, kind="ExternalInput")
    kv1_d = nc.dram_tensor("kv1", [P, S], f16, kind="ExternalInput")
    # kvx transposed [j within block, f], split jb 0-3 / 4-7
    kvta_d = nc.dram_tensor("kvta", [P, 4, F], f16, kind="ExternalInput")
    kvtb_d = nc.dram_tensor("kvtb", [P, 4, F], f16, kind="ExternalInput")
    # heads 1..7: [slot 0=A_h rows g | slot 1=N_h rows f][c][col],
    # head-major so each per-head DMA is one contiguous 256KB block
    wh_d = nc.dram_tensor("wh", [H - 1, P, 2, 2, F], f16, kind="ExternalInput")
    # chunk-major: each fo-chunk is one contiguous 128KB block in DRAM
    out_d = nc.dram_tensor("out", [2, P, HALF], f16, kind="ExternalOutput")

    with tile.TileContext(nc) as tc:
        with (
            tc.tile_pool(name="io", bufs=1) as io_pool,
            tc.tile_pool(name="wts", bufs=2) as w_pool,
            tc.tile_pool(name="qa", bufs=2) as qa_pool,
            tc.tile_pool(name="raw", bufs=2) as raw_pool,
            tc.tile_pool(name="attn", bufs=2) as attn_pool,
            tc.tile_pool(name="ps", bufs=6, space="PSUM") as ps_pool,
            tc.tile_pool(name="ops", bufs=1, space="PSUM") as out_ps_pool,
        ):
            fr = io_pool.tile([P, 2, F + HALF], f16, tag="fr")
            n0 = io_pool.tile([P, 2, F], f16, tag="n0")
            kvin = [
                io_pool.tile([P, S], f16, tag=f"kvin{c}", name=f"kvin{c}")
                for c in range(2)
            ]
            kvt = [
                io_pool.tile([P, 4, F], f16, tag=f"kvt{u}", name=f"kvt{u}")
                for u in range(2)
            ]
            # warm tile is deliberately NEVER initialized: warmup matmuls
            # only exist to keep the PE clock ramped, their results are
            # discarded, and skipping the memset lets warmup start at
            # block entry (~7.3us) with no cross-engine dependency.
            warm = io_pool.tile([P, HALF], dt.bfloat16, tag="warm")

            # front triggers, one engine each (each DIRECT2D costs ~0.65us
            # of issuing-engine time; data starts ~1.4us after trigger)
            nc.sync.dma_start(fr[:], fr_d[:])
            nc.sync.dma_start(n0[:], n0_d[:])
            nc.scalar.dma_start(kvin[0][:], kv0_d[:])
            nc.scalar.dma_start(kvt[0][:], kvta_d[:])
            nc.gpsimd.dma_start(kvin[1][:], kv1_d[:])
            nc.gpsimd.dma_start(kvt[1][:], kvtb_d[:])

            # heads 1..7 weights stream on gpsimd; bufs=2 pool holds each
            # DMA until head h-2's weights are consumed
            ws = [None] * H

            def fetch_w(h):
                ws[h] = w_pool.tile([P, 2, 2, F], f16, tag="w", name=f"w{h}")
                nc.gpsimd.dma_start(ws[h][:], wh_d[h - 1])

            fetch_w(1)
            fetch_w(2)

            # PE pre-warm: fine-grained dummy matmuls bridge until the
            # front DMAs land while keeping the PE clock ramped.
            nwarm = int(os.environ.get("ATTN_NWARM", "40"))
            wps = [
                ps_pool.tile([P, HALF], f32, tag="ps", name=f"wps{i}")
                for i in range(2)
            ]
            for i in range(nwarm):
                nc.tensor.matmul(
                    wps[i % 2][:, :P], warm[:, :P], warm[:, :P],
                    start=True, stop=True,
                )
            # tiny engine wake-ups anchored on the warm tile's WAR
            # dependency: they execute right at bridge-end, so both cast
            # engines are recently-active when head 0's casts arrive.
            nc.vector.tensor_copy(warm[:, :16], wps[0][:, :16])
            nc.scalar.activation(warm[:, 16:32], wps[1][:, :16], AF.Copy)

            # persistent accumulators for the folded output projection,
            # one tile per fo-chunk so each chunk's ReLU depends only on
            # its own final matmul (deps are tile-granular for writers)
            out_ps = [
                out_ps_pool.tile([P, HALF], f32, tag=f"out_ps{t}", name=f"out_ps{t}")
                for t in range(2)
            ]

            def a_slice(h, c, t):
                if h == 0:
                    return fr[:, c, P * t : P * (t + 1)]
                return ws[h][:, 0, c, P * t : P * (t + 1)]

            def n_slice(h, c, t):
                if h == 0:
                    return n0[:, c, P * t : P * (t + 1)]
                return ws[h][:, 1, c, P * t : P * (t + 1)]

            qin_ap = fr[:, :, F:]

            def qa_proj(h):
                """Emit q-side projection qa = A_h^T @ qin for head h."""
                qa = qa_pool.tile([P, 2, HALF], f16, tag="qa", name=f"qa{h}")
                for t in range(2):
                    ps = ps_pool.tile([P, HALF], f32, tag="ps", name=f"psqa{h}{t}")
                    for c in range(2):
                        nc.tensor.matmul(
                            ps[:],
                            a_slice(h, c, t),
                            qin_ap[:, c, :],
                            start=(c == 0),
                            stop=(c == 1),
                        )
                    # split casts across engines to halve qa latency
                    if t == 0:
                        nc.vector.tensor_copy(qa[:, t, :], ps[:])
                    else:
                        nc.scalar.activation(qa[:, t, :], ps[:], AF.Copy)
                return qa

            qa_next = qa_proj(0)
            for h in range(H):
                qa = qa_next

                # qktT (j 8x128, i 512) = kvin_slice^T @ qa; sigmoid on ACT
                atn = attn_pool.tile([P, 8, HALF], f16, tag="atn", name=f"atn{h}")
                pss = {}
                if h == 0:
                    # kv c1 rides the late gpsimd ring: open six c0-only
                    # chains (6 = whole ps pool) before touching c1
                    seq = [(jb, 0) for jb in range(6)]
                    seq += [(jb, 1) for jb in range(6)]
                    seq += [(6, 0), (6, 1), (7, 0), (7, 1)]
                else:
                    seq = [(jb, c) for jb in range(8) for c in range(2)]
                for jb, c in seq:
                    if c == 0:
                        pss[jb] = ps_pool.tile(
                            [P, HALF], f32, tag="ps", name=f"psq{h}{jb}"
                        )
                    nc.tensor.matmul(
                        pss[jb][:],
                        kvin[c][:, P * jb : P * (jb + 1)],
                        qa[:, c, :],
                        start=(c == 0),
                        stop=(c == 1),
                    )
                    if c == 1:
                        nc.scalar.activation(
                            atn[:, jb, :], pss[jb][:], AF.Sigmoid, scale=16.0
                        )

                # fetch head h+2's weights (held by the pool-reuse dep
                # until head h's weights are consumed)
                if 3 <= h + 2 < H:
                    fetch_w(h + 2)

                # rawT (fk 2x128, i 512) = sum_jb kvt_slice^T @ atn_jb.
                # Two chains (fk chunks) interleaved per jb so the PE
                # consumes each sigmoid output ~2x later than a straight
                # chain would.
                raw = raw_pool.tile([P, 2, HALF], f16, tag="raw", name=f"raw{h}")
                rps = [
                    ps_pool.tile([P, HALF], f32, tag="ps", name=f"psr{h}{t}")
                    for t in range(2)
                ]
                # finish the t=0 chain 3 matmuls early so its cast runs
                # under the t=1 tail and the N-apply (which consumes raw
                # chunk 0 first) never stalls on the serialized vector
                # casts. Sigmoid jb7 still lands ~0.7us before (7,0).
                seq = [(jb, t) for jb in range(5) for t in range(2)]
                seq += [(5, 0), (6, 0), (7, 0), (5, 1), (6, 1), (7, 1)]
                for jb, t in seq:
                    nc.tensor.matmul(
                        rps[t][:],
                        kvt[jb // 4][:, jb % 4, P * t : P * (t + 1)],
                        atn[:, jb, :],
                        start=(jb == 0),
                        stop=(jb == 7),
                    )
                # both casts on vector: scalar is sigmoid-backlogged here
                nc.vector.tensor_copy(raw[:, 0, :], rps[0][:])
                nc.vector.tensor_copy(raw[:, 1, :], rps[1][:])

                # software-pipeline: next head's qa projection here so the
                # PE has dependency-free work across the head boundary
                if h + 1 < H:
                    qa_next = qa_proj(h + 1)

                # outT (fo 2x128, i 512) += N_h^T @ rawT, persistent accum.
                last = h == H - 1
                if not last:
                    # c-outer: first two matmuls only need raw chunk 0
                    for c in range(2):
                        for t2 in range(2):
                            nc.tensor.matmul(
                                out_ps[t2][:],
                                n_slice(h, c, t2),
                                raw[:, c, :],
                                start=(h == 0 and c == 0),
                                stop=False,
                            )
                else:
                    # chunk 1 completes one matmul early so its ReLU +
                    # DMA (scalar path, slower semaphore) start under the
                    # final chunk-0 matmul
                    for c, t2 in ((0, 1), (0, 0), (1, 1), (1, 0)):
                        nc.tensor.matmul(
                            out_ps[t2][:],
                            n_slice(h, c, t2),
                            raw[:, c, :],
                            start=False,
                            stop=(c == 1),
                        )

            # tail: parallel ReLUs on vector+scalar; chunk 1 (done first)
            # splits its stream across the scalar+sync rings, chunk 0
            # (vector relu, ~40ns start) rides sync. Keep gpsimd out of
            # late DMAs: its dge_drain serializes ~2.5us behind in-flight
            # SWDGE transfers before the exit barrier.
            # separate SBUF tiles per chunk: writers serialize per tile,
            # so sharing one tile would chain the two ReLUs
            out_sb1 = io_pool.tile([P, HALF], f16, tag="out_sb1")
            nc.scalar.activation(out_sb1[:], out_ps[1][:], AF.Relu)
            nc.scalar.dma_start(out_d[1], out_sb1[:])
            out_sb0 = io_pool.tile([P, HALF], f16, tag="out_sb0")
            nc.vector.tensor_relu(out_sb0[:], out_ps[0][:])
            nc.sync.dma_start(out_d[0], out_sb0[:])

    nc.compile()
    return nc


def _get_nc():
    if "nc" not in _cache:
        _cache["nc"] = _build()
    return _cache["nc"]


def _make_in_maps(inputs):
    q_input = np.asarray(inputs["q_input"], dtype=np.float32)
    kv_input = np.asarray(inputs["kv_input"], dtype=np.float32)
    Wq4 = np.asarray(inputs["Wq"], dtype=np.float32).reshape(F, F, H)
    Wk4 = np.asarray(inputs["Wk"], dtype=np.float32).reshape(F, F, H)
    Wv4 = np.asarray(inputs["Wv"], dtype=np.float32).reshape(F, F, H)
    Wz3 = np.asarray(inputs["Wz"], dtype=np.float32).reshape(F, H, F)

    # host-side folds in fp32: A_h = Wq_h Wk_h^T, N_h = Wv_h Wz_h
    A = np.einsum("gdh,fdh->hgf", Wq4, Wk4, optimize=True)  # [H, g, fk]
    N = np.einsum("fdh,dho->hfo", Wv4, Wz3, optimize=True)  # [H, fk, fo]
    # [h, p, c, col] with contraction row = c*128+p
    A_pc = np.ascontiguousarray(
        A.reshape(H, 2, P, F).transpose(0, 2, 1, 3), dtype=np.float16
    )
    N_pc = np.ascontiguousarray(
        N.reshape(H, 2, P, F).transpose(0, 2, 1, 3), dtype=np.float16
    )
    # heads 1..7 packed [h-1, p, slot, c, col]
    WH = np.ascontiguousarray(
        np.stack([A_pc[1:], N_pc[1:]], axis=2), dtype=np.float16
    )  # [H-1, P, 2, 2, F]

    in_maps = []
    for core in range(NCORES):
        b, half = divmod(core, 2)
        qb = q_input[b].reshape(2, P, S)
        qin = qb[:, :, half * HALF : (half + 1) * HALF].transpose(1, 0, 2)
        fr = np.ascontiguousarray(
            np.concatenate([A_pc[0], qin.astype(np.float16)], axis=2),
            dtype=np.float16,
        )  # [P, 2, F+HALF]
        kvc = kv_input[b].reshape(2, P, S).astype(np.float16)
        # kvt[p, jb, f] = kv_input[b][f, jb*128+p]
        kvt = np.ascontiguousarray(
            kv_input[b].T.reshape(8, P, F).transpose(1, 0, 2), dtype=np.float16
        )
        in_maps.append(
            {
                "fr": fr,
                "n0": np.ascontiguousarray(N_pc[0]),
                "kv0": np.ascontiguousarray(kvc[0]),
                "kv1": np.ascontiguousarray(kvc[1]),
                "kvta": np.ascontiguousarray(kvt[:, 0:4]),
                "kvtb": np.ascontiguousarray(kvt[:, 4:8]),
                "wh": WH,
            }
        )
    return in_maps


def kernel(q_input, kv_input, Wq, Wk, Wv, Wz, **kw):
    from concourse.bass_utils import run_bass_kernel_spmd

    nc = _get_nc()
    in_maps = _make_in_maps(
        {
            "q_input": q_input,
            "kv_input": kv_input,
            "Wq": Wq,
            "Wk": Wk,
            "Wv": Wv,
            "Wz": Wz,
        }
    )

    res = run_bass_kernel_spmd(nc, in_maps, core_ids=list(range(NCORES)))

    out = np.empty((B, F, S), dtype=np.float32)
    for c in range(NCORES):
        b, half = divmod(c, 2)
        # out dram [chunk, p, i] -> out[b, chunk*128+p, half*512+i]
        o = np.asarray(res.results[c]["out"], dtype=np.float32)  # (2, P, HALF)
        out[b, :, half * HALF : (half + 1) * HALF] = o.reshape(F, HALF)
    return out


# revision 49
# speedup vs baseline: 1.0703x; 1.0136x over previous
"""Trainium2 Bass kernel for sigmoid-gated multi-head attention.

Reference computation (B=4, F=256, H=8, S=1024):
    qx  = q_input^T          (b, s, f)
    q   = qx @ Wq  -> (b, s, f, h)   [col fi*H + hi]
    k,v = kvx @ Wk / Wv
    attn = sigmoid(sqrt(F) * q.k)    per head
    wv   = attn @ v
    out  = relu(concat_heads(wv) @ Wz)   returned as (b, f, s)

Algebraic restructure (host-side weight folding):
    qkt_h = qx (Wq_h Wk_h^T) kvx^T = qx A_h kvx^T
    out   = relu(sum_h attn_h kvx (Wv_h Wz_h)) = relu(sum_h (attn_h kvx) N_h)
A_h and N_h are tiny 256x256 products computed on the host in fp32.

Per-core MACs 2.684G = the perfect 8-way-sharding ideal, zero
collectives. PE floor 68.3us at the fp16 1-col/cycle rate (measured:
fp8 DoubleRow retires the same 1 col/cycle with 2 K-tiles -> any
precision-preserving fp8 scheme needs hi+lo splits and only breaks
even, so fp16 is optimal here).

Sharding: 8 cores = 4 batches x 2 query-sequence halves; per-core
outputs are disjoint slices of the final output.

Front-window plan (runtime preamble fixed at ~7.2us; DMA triggers cost
~0.65us each, serialized per issuing engine; first ring data ~1.4us
after trigger):
    sync   ring: fr = [A0|qin] packed (384KB, one trigger), then n0
    scalar ring: kv_c0 (256KB) then kvt jb0-3 (256KB)
    gpsimd ring: kv_c1, kvt jb4-7, w1, w2, then A|N weight pairs for
                 heads 3..7 held by the bufs=2 pool-reuse dependency
                 until head h-2's weights are consumed
PE warmup (junk tile, deliberately uninitialized so nothing gates it)
runs from block entry so the clock is ramped when qa0 fires at
~10.5-11.5us (data-gated). Head-0 qkt opens six c0-only PSUM chains
first so late-landing kv_c1 (gpsimd ring) doesn't stall.

Per head (all matmuls [128 x (2|8 chained) x 512], fp16 operands,
fp32 PSUM):
    qa   (fk 2x128, i 512) = A_h^T @ qin           4 mm   (pipelined 1 head ahead)
    qktT (j 8x128, i 512)  = kvin^T_slice @ qa    16 mm -> sigmoid(16x) -> atn
    rawT (fk 2x128, i 512) = sum_jb kvt_slice^T @ atn
                                                  16 mm (2 interleaved chains)
    outT (fo 2x128, i 512) += N_h^T @ rawT         4 mm  (persistent PSUM accum)

Tail: final head's out matmuls ordered so chunk 1 completes first
(its scalar-relu path has ~0.8us semaphore latency); per-chunk out_ps
and out_sb tiles keep the two relu+DMA paths independent (writers
serialize per tile); both 128KB output chunks stream concurrently on
the sync and scalar HWDGE rings (never gpsimd: its dge_drain
serializes ~2.5us behind in-flight transfers before the exit
barrier). Measured last-matmul to last-byte: ~2.8us.
"""

import os
import sys

sys.path.insert(0, "/opt/trn_rl_repo")

import numpy as np

B, F, H, S = 4, 256, 8, 1024
HALF = S // 2  # query columns per core
QTR = HALF // 2
NCORES = 8
P = 128  # partitions

_cache = {}


def _build():
    import concourse.mybir as mybir
    import concourse.tile as tile
    from concourse import bacc

    dt = mybir.dt
    f32 = dt.float32
    f16 = dt.float16
    AF = mybir.ActivationFunctionType

    nc = bacc.Bacc(None, target_bir_lowering=False)

    # fr[p, c, 0:F]      = A0[c*128+p, :]   (head-0 q-side fold)
    # fr[p, c, F:F+HALF] = qin[c*128+p, :]  (this core's q slice)
    fr_d = nc.dram_tensor("fr", [P, 2, F + HALF], f16, kind="ExternalInput")
    n0_d = nc.dram_tensor("n0", [P, 2, F], f16, kind="ExternalInput")
    # kvin chunk c: [f-low partition, j]; two tensors so each rides its
    # own ring and head-0's c0-only chains can start before c1 lands.
    kv0_d = nc.dram_tensor("kv0", [P, S], f16, kind="ExternalInput")
    kv1_d = nc.dram_tensor("kv1", [P, S], f16, kind="ExternalInput")
    # kvx transposed [j within block, f], split jb 0-3 / 4-7
    kvta_d = nc.dram_tensor("kvta", [P, 4, F], f16, kind="ExternalInput")
    kvtb_d = nc.dram_tensor("kvtb", [P, 4, F], f16, kind="ExternalInput")
    # heads 1..7: [slot 0=A_h rows g | slot 1=N_h rows f][c][col],
    # head-major so each per-head DMA is one contiguous 256KB block
    wh_d = nc.dram_tensor("wh", [H - 1, P, 2, 2, F], f16, kind="ExternalInput")
    # chunk-major: each fo-chunk is one contiguous 128KB block in DRAM
    out_d = nc.dram_tensor("out", [2, P, HALF], f16, kind="ExternalOutput")

    with tile.TileContext(nc) as tc:
        with (
            tc.tile_pool(name="io", bufs=1) as io_pool,
            tc.tile_pool(name="wts", bufs=2) as w_pool,
            tc.tile_pool(name="qa", bufs=2) as qa_pool,
            tc.tile_pool(name="raw", bufs=2) as raw_pool,
            tc.tile_pool(name="attn", bufs=2) as attn_pool,
            tc.tile_pool(name="ps", bufs=6, space="PSUM") as ps_pool,
            tc.tile_pool(name="ops", bufs=1, space="PSUM") as out_ps_pool,
        ):
            fr = io_pool.tile([P, 2, F + HALF], f16, tag="fr")
            n0 = io_pool.tile([P, 2, F], f16, tag="n0")
            kvin = [
                io_pool.tile([P, S], f16, tag=f"kvin{c}", name=f"kvin{c}")
                for c in range(2)
            ]
            kvt = [
                io_pool.tile([P, 4, F], f16, tag=f"kvt{u}", name=f"kvt{u}")
                for u in range(2)
            ]
            # warm tile is deliberately NEVER initialized: warmup matmuls
            # only exist to keep the PE clock ramped, their results are
            # discarded, and skipping the memset lets warmup start at
            # block entry (~7.3us) with no cross-engine dependency.
            warm = io_pool.tile([P, HALF], dt.bfloat16, tag="warm")

            # front triggers, one engine each (each DIRECT2D costs ~0.65us
            # of issuing-engine time; data starts ~1.4us after trigger)
            nc.sync.dma_start(fr[:], fr_d[:])
            nc.sync.dma_start(n0[:], n0_d[:])
            nc.scalar.dma_start(kvin[0][:], kv0_d[:])
            nc.scalar.dma_start(kvt[0][:], kvta_d[:])
            nc.gpsimd.dma_start(kvin[1][:], kv1_d[:])
            nc.gpsimd.dma_start(kvt[1][:], kvtb_d[:])

            # heads 1..7 weights stream on gpsimd; bufs=2 pool holds each
            # DMA until head h-2's weights are consumed
            ws = [None] * H

            def fetch_w(h):
                ws[h] = w_pool.tile([P, 2, 2, F], f16, tag="w", name=f"w{h}")
                nc.gpsimd.dma_start(ws[h][:], wh_d[h - 1])

            fetch_w(1)
            fetch_w(2)

            # PE pre-warm: fine-grained dummy matmuls bridge until the
            # front DMAs land while keeping the PE clock ramped.
            nwarm = int(os.environ.get("ATTN_NWARM", "40"))
            wps = [
                ps_pool.tile([P, HALF], f32, tag="ps", name=f"wps{i}")
                for i in range(2)
            ]
            for i in range(nwarm):
                nc.tensor.matmul(
                    wps[i % 2][:, :P], warm[:, :P], warm[:, :P],
                    start=True, stop=True,
                )
            # tiny engine wake-ups anchored on the warm tile's WAR
            # dependency: they execute right at bridge-end, so both cast
            # engines are recently-active when head 0's casts arrive.
            # The scalar wake is a SIGMOID on purpose: as the first
            # scalar activation it makes the ACT-table pass load a
            # sigmoid table (which also covers Copy/Relu/Identity) once,
            # instead of a copy table first plus a second 1.28us load
            # right before head 0's sigmoid stream (that load measurably
            # starved the head-0/1 sigmoids, ~1.5us of PE stalls).
            nc.vector.tensor_copy(warm[:, :16], wps[0][:, :16])
            nc.scalar.activation(warm[:, 16:32], wps[1][:, :16], AF.Sigmoid)

            # persistent accumulators for the folded output projection,
            # one tile per fo-chunk so each chunk's ReLU depends only on
            # its own final matmul (deps are tile-granular for writers)
            out_ps = [
                out_ps_pool.tile([P, HALF], f32, tag=f"out_ps{t}", name=f"out_ps{t}")
                for t in range(2)
            ]

            def a_slice(h, c, t):
                if h == 0:
                    return fr[:, c, P * t : P * (t + 1)]
                return ws[h][:, 0, c, P * t : P * (t + 1)]

            def n_slice(h, c, t):
                if h == 0:
                    return n0[:, c, P * t : P * (t + 1)]
                return ws[h][:, 1, c, P * t : P * (t + 1)]

            qin_ap = fr[:, :, F:]

            def qa_proj(h):
                """Emit q-side projection qa = A_h^T @ qin for head h."""
                qa = qa_pool.tile([P, 2, HALF], f16, tag="qa", name=f"qa{h}")
                for t in range(2):
                    ps = ps_pool.tile([P, HALF], f32, tag="ps", name=f"psqa{h}{t}")
                    for c in range(2):
                        nc.tensor.matmul(
                            ps[:],
                            a_slice(h, c, t),
                            qin_ap[:, c, :],
                            start=(c == 0),
                            stop=(c == 1),
                        )
                    # split casts across engines to halve qa latency
                    if t == 0:
                        nc.vector.tensor_copy(qa[:, t, :], ps[:])
                    else:
                        nc.scalar.activation(qa[:, t, :], ps[:], AF.Copy)
                return qa

            qa_next = qa_proj(0)
            for h in range(H):
                qa = qa_next

                # qktT (j 8x128, i 512) = kvin_slice^T @ qa; sigmoid on ACT
                atn = attn_pool.tile([P, 8, HALF], f16, tag="atn", name=f"atn{h}")
                pss = {}
                if h == 0:
                    # kv c1 rides the late gpsimd ring: open six c0-only
                    # chains (6 = whole ps pool) before touching c1
                    seq = [(jb, 0) for jb in range(6)]
                    seq += [(jb, 1) for jb in range(6)]
                    seq += [(6, 0), (6, 1), (7, 0), (7, 1)]
                else:
                    seq = [(jb, c) for jb in range(8) for c in range(2)]
                for jb, c in seq:
                    if c == 0:
                        pss[jb] = ps_pool.tile(
                            [P, HALF], f32, tag="ps", name=f"psq{h}{jb}"
                        )
                    nc.tensor.matmul(
                        pss[jb][:],
                        kvin[c][:, P * jb : P * (jb + 1)],
                        qa[:, c, :],
                        start=(c == 0),
                        stop=(c == 1),
                    )
                    if c == 1:
                        nc.scalar.activation(
                            atn[:, jb, :], pss[jb][:], AF.Sigmoid, scale=16.0
                        )

                # fetch head h+2's weights (held by the pool-reuse dep
                # until head h's weights are consumed)
                if 3 <= h + 2 < H:
                    fetch_w(h + 2)

                # rawT (fk 2x128, i 512) = sum_jb kvt_slice^T @ atn_jb.
                # Two chains (fk chunks) interleaved per jb so the PE
                # consumes each sigmoid output ~2x later than a straight
                # chain would.
                raw = raw_pool.tile([P, 2, HALF], f16, tag="raw", name=f"raw{h}")
                rps = [
                    ps_pool.tile([P, HALF], f32, tag="ps", name=f"psr{h}{t}")
                    for t in range(2)
                ]
                # finish the t=0 chain 3 matmuls early so its cast runs
                # under the t=1 tail and the N-apply (which consumes raw
                # chunk 0 first) never stalls on the serialized vector
                # casts. Sigmoid jb7 still lands ~0.7us before (7,0).
                seq = [(jb, t) for jb in range(5) for t in range(2)]
                seq += [(5, 0), (6, 0), (7, 0), (5, 1), (6, 1), (7, 1)]
                for jb, t in seq:
                    nc.tensor.matmul(
                        rps[t][:],
                        kvt[jb // 4][:, jb % 4, P * t : P * (t + 1)],
                        atn[:, jb, :],
                        start=(jb == 0),
                        stop=(jb == 7),
                    )
                # both casts on vector: scalar is sigmoid-backlogged here
                nc.vector.tensor_copy(raw[:, 0, :], rps[0][:])
                nc.vector.tensor_copy(raw[:, 1, :], rps[1][:])

                # software-pipeline: next head's qa projection here so the
                # PE has dependency-free work across the head boundary
                if h + 1 < H:
                    qa_next = qa_proj(h + 1)

                # outT (fo 2x128, i 512) += N_h^T @ rawT, persistent accum.
                last = h == H - 1
                if not last:
                    # c-outer: first two matmuls only need raw chunk 0
                    for c in range(2):
                        for t2 in range(2):
                            nc.tensor.matmul(
                                out_ps[t2][:],
                                n_slice(h, c, t2),
                                raw[:, c, :],
                                start=(h == 0 and c == 0),
                                stop=False,
                            )
                else:
                    # chunk 1 completes one matmul early so its ReLU +
                    # DMA (scalar path, slower semaphore) start under the
                    # final chunk-0 matmul
                    for c, t2 in ((0, 1), (0, 0), (1, 1), (1, 0)):
                        nc.tensor.matmul(
                            out_ps[t2][:],
                            n_slice(h, c, t2),
                            raw[:, c, :],
                            start=False,
                            stop=(c == 1),
                        )

            # tail: parallel ReLUs on vector+scalar; chunk 1 (done first)
            # splits its stream across the scalar+sync rings, chunk 0
            # (vector relu, ~40ns start) rides sync. Keep gpsimd out of
            # late DMAs: its dge_drain serializes ~2.5us behind in-flight
            # SWDGE transfers before the exit barrier.
            # separate SBUF tiles per chunk: writers serialize per tile,
            # so sharing one tile would chain the two ReLUs
            out_sb1 = io_pool.tile([P, HALF], f16, tag="out_sb1")
            nc.scalar.activation(out_sb1[:], out_ps[1][:], AF.Relu)
            nc.scalar.dma_start(out_d[1], out_sb1[:])
            out_sb0 = io_pool.tile([P, HALF], f16, tag="out_sb0")
            nc.vector.tensor_relu(out_sb0[:], out_ps[0][:])
            nc.sync.dma_start(out_d[0], out_sb0[:])

    nc.compile()
    return nc


def _get_nc():
    if "nc" not in _cache:
        _cache["nc"] = _build()
    return _cache["nc"]


def _make_in_maps(inputs):
    q_input = np.asarray(inputs["q_input"], dtype=np.float32)
    kv_input = np.asarray(inputs["kv_input"], dtype=np.float32)
    Wq4 = np.asarray(inputs["Wq"], dtype=np.float32).reshape(F, F, H)
    Wk4 = np.asarray(inputs["Wk"], dtype=np.float32).reshape(F, F, H)
    Wv4 = np.asarray(inputs["Wv"], dtype=np.float32).reshape(F, F, H)
    Wz3 = np.asarray(inputs["Wz"], dtype=np.float32).reshape(F, H, F)

    # host-side folds in fp32: A_h = Wq_h Wk_h^T, N_h = Wv_h Wz_h
    A = np.einsum("gdh,fdh->hgf", Wq4, Wk4, optimize=True)  # [H, g, fk]
    N = np.einsum("fdh,dho->hfo", Wv4, Wz3, optimize=True)  # [H, fk, fo]
    # [h, p, c, col] with contraction row = c*128+p
    A_pc = np.ascontiguousarray(
        A.reshape(H, 2, P, F).transpose(0, 2, 1, 3), dtype=np.float16
    )
    N_pc = np.ascontiguousarray(
        N.reshape(H, 2, P, F).transpose(0, 2, 1, 3), dtype=np.float16
    )
    # heads 1..7 packed [h-1, p, slot, c, col]
    WH = np.ascontiguousarray(
        np.stack([A_pc[1:], N_pc[1:]], axis=2), dtype=np.float16
    )  # [H-1, P, 2, 2, F]

    in_maps = []
    for core in range(NCORES):
        b, half = divmod(core, 2)
        qb = q_input[b].reshape(2, P, S)
        qin = qb[:, :, half * HALF : (half + 1) * HALF].transpose(1, 0, 2)
        fr = np.ascontiguousarray(
            np.concatenate([A_pc[0], qin.astype(np.float16)], axis=2),
            dtype=np.float16,
        )  # [P, 2, F+HALF]
        kvc = kv_input[b].reshape(2, P, S).astype(np.float16)
        # kvt[p, jb, f] = kv_input[b][f, jb*128+p]
        kvt = np.ascontiguousarray(
            kv_input[b].T.reshape(8, P, F).transpose(1, 0, 2), dtype=np.float16
        )
        in_maps.append(
            {
                "fr": fr,
                "n0": np.ascontiguousarray(N_pc[0]),
                "kv0": np.ascontiguousarray(kvc[0]),
                "kv1": np.ascontiguousarray(kvc[1]),
                "kvta": np.ascontiguousarray(kvt[:, 0:4]),
                "kvtb": np.ascontiguousarray(kvt[:, 4:8]),
                "wh": WH,
            }
        )
    return in_maps


def kernel(q_input, kv_input, Wq, Wk, Wv, Wz, **kw):
    from concourse.bass_utils import run_bass_kernel_spmd

    nc = _get_nc()
    in_maps = _make_in_maps(
        {
            "q_input": q_input,
            "kv_input": kv_input,
            "Wq": Wq,
            "Wk": Wk,
            "Wv": Wv,
            "Wz": Wz,
        }
    )

    res = run_bass_kernel_spmd(nc, in_maps, core_ids=list(range(NCORES)))

    out = np.empty((B, F, S), dtype=np.float32)
    for c in range(NCORES):
        b, half = divmod(c, 2)
        # out dram [chunk, p, i] -> out[b, chunk*128+p, half*512+i]
        o = np.asarray(res.results[c]["out"], dtype=np.float32)  # (2, P, HALF)
        out[b, :, half * HALF : (half + 1) * HALF] = o.reshape(F, HALF)
    return out
